# revision 1
# baseline (speedup 1.0000x reference)
"""KAN layer kernel for TRN2, 8-core SPMD.

Math: out[b,o] = sum_{i,k} relu(x[b,i]*w1[o,i,k] + b1[o,i,k]) * w2[o,i,k] / 32 + b2[o]
With b1 == 0 (guaranteed by the generator) the relu factorizes:
    relu(x*w) = max(x,0)*max(w,0) + min(x,0)*min(w,0)
so the whole layer collapses to two matmuls with preprocessed weights:
    Ap[o,i] = sum_k max(w1,0)*w2      Am[o,i] = sum_k min(w1,0)*w2
    out = (max(x,0) @ Ap^T + min(x,0) @ Am^T) / 32 + b2

Sharding: 4 batch groups x 2 dout groups (core = bi*2 + oj).
Per core: x^T shard [256, 512], w1/w2 slabs [256, 4, 128] (din, k, dout-slab),
all weight preprocessing done on-device; output is out^T [128, 512].
"""

import numpy as np

B, DIN, DOUT, K = 2048, 256, 256, 4
N_CORES = 8
BG, OG = 4, 2                      # batch groups x dout groups
BS, OS = B // BG, DOUT // OG       # 512 batch rows, 128 dout cols per core
SCALE = 1.0 / np.sqrt(((DOUT + DIN) / 2) * K)   # 1/32

_CACHE = {}


def _build_nc():
    if "nc" in _CACHE:
        return _CACHE["nc"]
    import concourse.bacc as bacc
    import concourse.tile as tile
    from concourse import mybir

    f32 = mybir.dt.float32
    nc = bacc.Bacc("TRN2", target_bir_lowering=False, debug=False,
                   num_devices=N_CORES)
    xt = nc.dram_tensor("xt", [DIN, BS], f32, kind="ExternalInput")
    w1t = nc.dram_tensor("w1t", [DIN, K, OS], f32, kind="ExternalInput")
    w2t = nc.dram_tensor("w2t", [DIN, K, OS], f32, kind="ExternalInput")
    b2s = nc.dram_tensor("b2s", [OS, 1], f32, kind="ExternalInput")
    outt = nc.dram_tensor("outt", [OS, BS], f32, kind="ExternalOutput")

    AF = mybir.ActivationFunctionType
    OP = mybir.AluOpType
    NT = DIN // 128                 # i-tiles

    with tile.TileContext(nc) as tc:
        with (
            tc.tile_pool(name="io", bufs=1) as io,
            tc.tile_pool(name="work", bufs=1) as work,
            tc.tile_pool(name="pp", bufs=1, space="PSUM") as pp,
        ):
            x_t, w1_t, w2_t = [], [], []
            HB = BS // 2
            for t in range(NT):
                w1i = io.tile([128, K, OS], f32, tag=f"w1{t}")
                nc.sync.dma_start(out=w1i, in_=w1t[t * 128:(t + 1) * 128, :, :])
                w1_t.append(w1i)
                w2i = io.tile([128, K, OS], f32, tag=f"w2{t}")
                nc.sync.dma_start(out=w2i, in_=w2t[t * 128:(t + 1) * 128, :, :])
                w2_t.append(w2i)
            # x halves: [i-tile][half], ordered so half 0 lands first
            xh = [[None, None] for _ in range(NT)]
            for h in range(2):
                for t in range(NT):
                    xi = io.tile([128, HB], f32, tag=f"x{t}{h}")
                    nc.sync.dma_start(
                        out=xi,
                        in_=xt[t * 128:(t + 1) * 128, h * HB:(h + 1) * HB])
                    xh[t][h] = xi
            b2_sb = io.tile([OS, 1], f32)
            nc.sync.dma_start(out=b2_sb, in_=b2s[:, :])

            # weight prep (DVE): ap = sum_k max(w1,0)*w2, amn = -sum_k min(w1,0)*w2
            ap_t, amn_t = [], []
            for t in range(NT):
                mpt = work.tile([128, K, OS], f32, tag=f"mp{t}")
                nc.vector.scalar_tensor_tensor(mpt, w1_t[t], 0.0, w2_t[t],
                                               op0=OP.max, op1=OP.mult)
                mmt = work.tile([128, K, OS], f32, tag=f"mm{t}")
                nc.vector.scalar_tensor_tensor(mmt, w1_t[t], 0.0, w2_t[t],
                                               op0=OP.min, op1=OP.mult)
                ap2 = work.tile([128, 2, OS], f32, tag=f"ap2{t}")
                nc.vector.tensor_add(ap2, mpt[:, 0:2, :], mpt[:, 2:4, :])
                ap = work.tile([128, OS], f32, tag=f"ap{t}")
                nc.vector.tensor_add(ap, ap2[:, 0, :], ap2[:, 1, :])
                am2 = work.tile([128, 2, OS], f32, tag=f"am2{t}")
                nc.vector.tensor_add(am2, mmt[:, 0:2, :], mmt[:, 2:4, :])
                amn = work.tile([128, OS], f32, tag=f"amn{t}")
                nc.vector.scalar_tensor_tensor(amn, am2[:, 0, :], -1.0,
                                               am2[:, 1, :],
                                               op0=OP.mult, op1=OP.subtract)
                ap_t.append(ap)
                amn_t.append(amn)

            # per-half: relu split (ACT) -> 4 matmuls -> epilogue -> store,
            # so half 0's tail hides under half 1's matmuls
            for h in range(2):
                psum = pp.tile([128, HB], f32, tag=f"ps{h}")
                mm = 0
                for t in range(NT):
                    xp = work.tile([128, HB], f32, tag=f"xp{t}{h}")
                    nc.scalar.activation(xp, xh[t][h], AF.Relu)
                    xn = work.tile([128, HB], f32, tag=f"xn{t}{h}")
                    nc.scalar.activation(xn, xh[t][h], AF.Relu, scale=-1.0)
                    nc.tensor.matmul(psum, lhsT=ap_t[t], rhs=xp,
                                     start=(mm == 0), stop=False)
                    mm += 1
                    nc.tensor.matmul(psum, lhsT=amn_t[t], rhs=xn,
                                     start=False, stop=(mm == 2 * NT - 1))
                    mm += 1
                out_sb = work.tile([128, HB], f32, tag=f"out{h}")
                nc.scalar.activation(out_sb, psum, AF.Identity,
                                     bias=b2_sb, scale=float(SCALE))
                nc.sync.dma_start(out=outt[:, h * HB:(h + 1) * HB], in_=out_sb)

    nc.compile()
    _CACHE["nc"] = nc
    return nc


def _kan_numpy(x, w1, b1, w2, b2):
    # exact fallback, chunked over batch to bound memory
    out = np.empty((x.shape[0], w1.shape[0]), dtype=np.float32)
    d = (w1.shape[0] + w1.shape[1]) / 2
    s = 1.0 / np.sqrt(d * w1.shape[2])
    for lo in range(0, x.shape[0], 128):
        hi = min(lo + 128, x.shape[0])
        h = x[lo:hi, None, :, None] * w1[None] + b1[None]
        np.maximum(h, 0.0, out=h)
        out[lo:hi] = np.einsum("boik,oik->bo", h, w2) * s
    return out + b2[None, :]


def kernel(x, w1, b1, w2, b2):
    x = np.ascontiguousarray(x, dtype=np.float32)
    w1 = np.asarray(w1, dtype=np.float32)
    b1 = np.asarray(b1, dtype=np.float32)
    w2 = np.asarray(w2, dtype=np.float32)
    b2 = np.asarray(b2, dtype=np.float32)

    if x.shape != (B, DIN) or w1.shape != (DOUT, DIN, K) or np.any(b1):
        return _kan_numpy(x, w1, b1, w2, b2)

    from concourse.bass_utils import run_bass_kernel_spmd

    nc = _build_nc()

    xT = np.ascontiguousarray(x.T)                      # (DIN, B)
    w1T = np.ascontiguousarray(w1.transpose(1, 2, 0))   # (DIN, K, DOUT)
    w2T = np.ascontiguousarray(w2.transpose(1, 2, 0))

    in_maps = []
    for core in range(N_CORES):
        bi, oj = divmod(core, OG)
        in_maps.append({
            "xt": np.ascontiguousarray(xT[:, bi * BS:(bi + 1) * BS]),
            "w1t": np.ascontiguousarray(w1T[:, :, oj * OS:(oj + 1) * OS]),
            "w2t": np.ascontiguousarray(w2T[:, :, oj * OS:(oj + 1) * OS]),
            "b2s": np.ascontiguousarray(b2[oj * OS:(oj + 1) * OS]).reshape(OS, 1),
        })

    res = run_bass_kernel_spmd(nc, in_maps, core_ids=list(range(N_CORES)))

    out = np.empty((B, DOUT), dtype=np.float32)
    for core in range(N_CORES):
        bi, oj = divmod(core, OG)
        out[bi * BS:(bi + 1) * BS, oj * OS:(oj + 1) * OS] = res.results[core]["outt"].T
    return out



# revision 3
# speedup vs baseline: 1.4275x; 1.4275x over previous
"""KAN layer kernel for TRN2, 8-core SPMD.

Math: out[b,o] = sum_{i,k} relu(x[b,i]*w1[o,i,k] + b1[o,i,k]) * w2[o,i,k] / 32 + b2[o]
With b1 == 0 (guaranteed by the generator) the relu factorizes:
    relu(x*w) = max(x,0)*max(w,0) + min(x,0)*min(w,0)
and with relu(-x) = relu(x) - x the layer collapses to two matmuls:
    S1[o,i] = sum_k max(w1,0)*w2     S2[o,i] = sum_k min(w1,0)*w2
    out = (relu(x) @ (S1-S2)^T + x @ S2^T) / 32 + b2

Sharding: 4 batch groups x 2 dout groups (core = bi*2 + oj).
All tensors staged host-side in bf16 (halves DMA bytes, 1 cycle/row
matmuls, 2x DVE); accumulation in fp32 PSUM; output fp32.
"""

import numpy as np

B, DIN, DOUT, K = 2048, 256, 256, 4
N_CORES = 8
BG, OG = 4, 2                      # batch groups x dout groups
BS, OS = B // BG, DOUT // OG       # 512 batch rows, 128 dout cols per core
NT = DIN // 128                    # din tiles
HB = BS // 2                       # psum half of the batch shard
SCALE = 1.0 / np.sqrt(((DOUT + DIN) / 2) * K)   # 1/32
N_WARM = 14                        # PE warm-up matmuls

_CACHE = {}


def _build_nc():
    if "nc" in _CACHE:
        return _CACHE["nc"]
    import concourse.bacc as bacc
    import concourse.tile as tile
    from concourse import mybir

    f32 = mybir.dt.float32
    bf16 = mybir.dt.bfloat16
    AF = mybir.ActivationFunctionType
    OP = mybir.AluOpType

    nc = bacc.Bacc("TRN2", target_bir_lowering=False, debug=False,
                   num_devices=N_CORES)
    # [i-part, i-tile, {w1,w2}, k, o]
    wt = nc.dram_tensor("wt", [128, NT, 2, K, OS], bf16, kind="ExternalInput")
    # [i-part, i-tile, b]
    xt = nc.dram_tensor("xt", [128, NT, BS], bf16, kind="ExternalInput")
    b2s = nc.dram_tensor("b2s", [OS, 1], f32, kind="ExternalInput")
    outt = nc.dram_tensor("outt", [OS, BS], f32, kind="ExternalOutput")

    with tile.TileContext(nc) as tc:
        with (
            tc.tile_pool(name="io", bufs=1) as io,
            tc.tile_pool(name="work", bufs=1) as work,
            tc.tile_pool(name="pp", bufs=1, space="PSUM") as pp,
        ):
            # ---- DMA in: weights first (feed prep), then x, then b2
            w_sb, x_sb = [], []
            for t in range(NT):
                wi = io.tile([128, 2, K, OS], bf16, tag=f"w{t}")
                nc.sync.dma_start(out=wi, in_=wt[:, t])
                w_sb.append(wi)
            for t in range(NT):
                xi = io.tile([128, BS], bf16, tag=f"x{t}")
                nc.sync.dma_start(out=xi, in_=xt[:, t])
                x_sb.append(xi)
            b2_sb = io.tile([OS, 1], f32)
            nc.sync.dma_start(out=b2_sb, in_=b2s[:, :])

            # ---- PE warm-up: keep the tensor engine busy from t~0 so the
            # real matmuls run at full clock (p-state ramp needs ~3us).
            zt = work.tile([128, 512], bf16, tag="zt")
            nc.gpsimd.memset(zt, 0.0)
            pz = pp.tile([128, 512], f32, tag="pz")
            for i in range(N_WARM):
                nc.tensor.matmul(pz, lhsT=zt[:, 0:128], rhs=zt,
                                 start=True, stop=True)

            # ---- weight prep (DVE), concat layout shares the k-sum adds:
            # cat[:, k, 0, :] = max(w1,0)*w2 ; cat[:, k, 1, :] = min(w1,0)*w2
            g_t, h_t = [], []
            for t in range(NT):
                cat = work.tile([128, K, 2, OS], bf16, tag=f"cat{t}")
                nc.vector.scalar_tensor_tensor(
                    cat[:, :, 0, :], w_sb[t][:, 0], 0.0, w_sb[t][:, 1],
                    op0=OP.max, op1=OP.mult)
                nc.vector.scalar_tensor_tensor(
                    cat[:, :, 1, :], w_sb[t][:, 0], 0.0, w_sb[t][:, 1],
                    op0=OP.min, op1=OP.mult)
                s2 = work.tile([128, 2, 2, OS], bf16, tag=f"s2{t}")
                nc.vector.tensor_add(s2, cat[:, 0:2], cat[:, 2:4])
                s = work.tile([128, 2, OS], bf16, tag=f"s{t}")
                nc.vector.tensor_add(s, s2[:, 0], s2[:, 1])
                g = work.tile([128, OS], bf16, tag=f"g{t}")
                nc.vector.tensor_sub(g, s[:, 0, :], s[:, 1, :])
                g_t.append(g)
                h_t.append(s[:, 1, :])

            # ---- relu(x): tile 0 on ACT, tile 1 on DVE (after prep)
            xp = []
            for t in range(NT):
                xpt = work.tile([128, BS], bf16, tag=f"xp{t}")
                xp.append(xpt)
            nc.scalar.activation(xp[0], x_sb[0], AF.Relu)
            nc.vector.tensor_scalar_max(xp[1], x_sb[1], 0.0)

            # ---- matmuls: psum half = batch half; weight-tile-0 matmuls
            # for both halves first so tile-1 prep overlaps them.
            psum = []
            for h in range(2):
                ps = pp.tile([128, HB], f32, tag=f"ps{h}")
                psum.append(ps)
            for t in range(NT):
                for h in range(2):
                    sl = slice(h * HB, (h + 1) * HB)
                    nc.tensor.matmul(psum[h], lhsT=g_t[t], rhs=xp[t][:, sl],
                                     start=(t == 0), stop=False)
                    nc.tensor.matmul(psum[h], lhsT=h_t[t], rhs=x_sb[t][:, sl],
                                     start=False, stop=(t == NT - 1))

            # ---- epilogue + store per half
            for h in range(2):
                out_sb = work.tile([128, HB], f32, tag=f"out{h}")
                nc.scalar.activation(out_sb, psum[h], AF.Identity,
                                     bias=b2_sb, scale=float(SCALE))
                nc.sync.dma_start(out=outt[:, h * HB:(h + 1) * HB], in_=out_sb)

    nc.compile()
    _CACHE["nc"] = nc
    return nc


def _kan_numpy(x, w1, b1, w2, b2):
    # exact fallback, chunked over batch to bound memory
    out = np.empty((x.shape[0], w1.shape[0]), dtype=np.float32)
    d = (w1.shape[0] + w1.shape[1]) / 2
    s = 1.0 / np.sqrt(d * w1.shape[2])
    for lo in range(0, x.shape[0], 128):
        hi = min(lo + 128, x.shape[0])
        h = x[lo:hi, None, :, None] * w1[None] + b1[None]
        np.maximum(h, 0.0, out=h)
        out[lo:hi] = np.einsum("boik,oik->bo", h, w2) * s
    return out + b2[None, :]


def kernel(x, w1, b1, w2, b2):
    x = np.asarray(x, dtype=np.float32)
    w1 = np.asarray(w1, dtype=np.float32)
    b1 = np.asarray(b1, dtype=np.float32)
    w2 = np.asarray(w2, dtype=np.float32)
    b2 = np.asarray(b2, dtype=np.float32)

    if x.shape != (B, DIN) or w1.shape != (DOUT, DIN, K) or np.any(b1):
        return _kan_numpy(x, w1, b1, w2, b2)

    import ml_dtypes
    from concourse.bass_utils import run_bass_kernel_spmd

    nc = _build_nc()
    bf16 = ml_dtypes.bfloat16

    xT = np.ascontiguousarray(x.T).astype(bf16)          # (DIN, B)
    w1T = w1.transpose(1, 2, 0).astype(bf16)             # (DIN, K, DOUT)
    w2T = w2.transpose(1, 2, 0).astype(bf16)

    in_maps = []
    for core in range(N_CORES):
        bi, oj = divmod(core, OG)
        osl = slice(oj * OS, (oj + 1) * OS)
        wt = np.empty((128, NT, 2, K, OS), dtype=bf16)
        for t in range(NT):
            isl = slice(t * 128, (t + 1) * 128)
            wt[:, t, 0] = w1T[isl, :, osl]
            wt[:, t, 1] = w2T[isl, :, osl]
        xtc = np.empty((128, NT, BS), dtype=bf16)
        for t in range(NT):
            xtc[:, t] = xT[t * 128:(t + 1) * 128, bi * BS:(bi + 1) * BS]
        in_maps.append({
            "wt": wt,
            "xt": xtc,
            "b2s": np.ascontiguousarray(b2[osl], dtype=np.float32).reshape(OS, 1),
        })

    res = run_bass_kernel_spmd(nc, in_maps, core_ids=list(range(N_CORES)))

    out = np.empty((B, DOUT), dtype=np.float32)
    for core in range(N_CORES):
        bi, oj = divmod(core, OG)
        out[bi * BS:(bi + 1) * BS, oj * OS:(oj + 1) * OS] = res.results[core]["outt"].T
    return out


# revision 4
# speedup vs baseline: 1.5168x; 1.0625x over previous
"""KAN layer kernel for TRN2, 8-core SPMD.

Math: out[b,o] = sum_{i,k} relu(x[b,i]*w1[o,i,k] + b1[o,i,k]) * w2[o,i,k] / 32 + b2[o]
With b1 == 0 (guaranteed by the generator) the relu factorizes:
    relu(x*w) = max(x,0)*max(w,0) + min(x,0)*min(w,0)
and with relu(-x) = relu(x) - x the layer collapses to two matmuls:
    T[o,i]  = sum_k w1*w2          S1[o,i] = sum_k relu(w1)*w2
    H = T - S1 ; G = S1 - H
    out = (relu(x) @ G^T + x @ H^T) / 32 + b2

Sharding: 4 batch groups x 2 dout groups (core = bi*2 + oj).
All tensors staged host-side in bf16 (halves DMA bytes, 1 cycle/row
matmuls, 2-4x DVE); accumulation in fp32 PSUM; bf16 output upcast on host.
relu(w1) runs at 4x DVE (tensor_scalar_max); one broadcast tensor_mul
computes w1*w2 and relu(w1)*w2 together at 2x.
"""

import numpy as np

B, DIN, DOUT, K = 2048, 256, 256, 4
N_CORES = 8
BG, OG = 4, 2                      # batch groups x dout groups
BS, OS = B // BG, DOUT // OG       # 512 batch rows, 128 dout cols per core
NT = DIN // 128                    # din tiles
HB = BS // 2                       # psum half of the batch shard
SCALE = 1.0 / np.sqrt(((DOUT + DIN) / 2) * K)   # 1/32
N_WARM = 36                        # PE warm-up matmuls ([128,128] each)

_CACHE = {}


def _build_nc():
    if "nc" in _CACHE:
        return _CACHE["nc"]
    import concourse.bacc as bacc
    import concourse.tile as tile
    from concourse import mybir

    f32 = mybir.dt.float32
    bf16 = mybir.dt.bfloat16
    AF = mybir.ActivationFunctionType
    OP = mybir.AluOpType

    nc = bacc.Bacc("TRN2", target_bir_lowering=False, debug=False,
                   num_devices=N_CORES)
    # [i-part, i-tile, {w1,w2}, k, o]
    wt = nc.dram_tensor("wt", [128, NT, 2, K, OS], bf16, kind="ExternalInput")
    # [i-part, i-tile, b]
    xt = nc.dram_tensor("xt", [128, NT, BS], bf16, kind="ExternalInput")
    b2s = nc.dram_tensor("b2s", [OS, 1], f32, kind="ExternalInput")
    outt = nc.dram_tensor("outt", [OS, BS], bf16, kind="ExternalOutput")

    with tile.TileContext(nc) as tc:
        with (
            tc.tile_pool(name="io", bufs=1) as io,
            tc.tile_pool(name="work", bufs=1) as work,
            tc.tile_pool(name="pp", bufs=1, space="PSUM") as pp,
        ):
            # ---- DMA in: weights first (feed prep), then x, then b2
            w_sb, x_sb = [], []
            for t in range(NT):
                # slots: 0=w1, 1=w2, 2=relu(w1) (filled on device)
                wi = io.tile([128, 3, K, OS], bf16, tag=f"w{t}")
                nc.sync.dma_start(out=wi[:, 0:2], in_=wt[:, t])
                w_sb.append(wi)
            for t in range(NT):
                xi = io.tile([128, BS], bf16, tag=f"x{t}")
                nc.sync.dma_start(out=xi, in_=xt[:, t])
                x_sb.append(xi)
            b2_sb = io.tile([OS, 1], f32)
            nc.sync.dma_start(out=b2_sb, in_=b2s[:, :])

            # ---- PE warm-up: keep the tensor engine busy from t~0 so the
            # real matmuls run at full clock (p-state ramp needs ~3us busy).
            zt = work.tile([128, 128], bf16, tag="zt")
            nc.gpsimd.memset(zt, 0.0)
            pz = pp.tile([128, 128], f32, tag="pz")
            for i in range(N_WARM):
                nc.tensor.matmul(pz, lhsT=zt, rhs=zt, start=True, stop=True)

            # ---- weight prep (DVE):
            #   w1p = relu(w1)                       (tensor_scalar_max, 4x)
            #   cat[:,0]=w1*w2, cat[:,1]=w1p*w2      (one broadcast mul, 2x)
            #   k-sum -> s[:,0]=T, s[:,1]=S1
            #   H = T - S1 ; G = S1 - H
            g_t, h_t = [], []
            for t in range(NT):
                wi = w_sb[t]
                nc.vector.tensor_scalar_max(wi[:, 2], wi[:, 0], 0.0)
                cat = work.tile([128, 2, K, OS], bf16, tag=f"cat{t}")
                in0 = wi[:, 0::2]
                in1 = wi[:, 1].unsqueeze(1).broadcast_to([128, 2, K, OS])
                nc.vector.tensor_mul(cat, in0, in1)
                s2 = work.tile([128, 2, 2, OS], bf16, tag=f"s2{t}")
                nc.vector.tensor_add(s2, cat[:, :, 0:2], cat[:, :, 2:4])
                s = work.tile([128, 2, OS], bf16, tag=f"s{t}")
                nc.vector.tensor_add(s, s2[:, :, 0], s2[:, :, 1])
                h = work.tile([128, OS], bf16, tag=f"h{t}")
                nc.vector.tensor_sub(h, s[:, 0], s[:, 1])
                g = work.tile([128, OS], bf16, tag=f"g{t}")
                nc.vector.tensor_sub(g, s[:, 1], h)
                g_t.append(g)
                h_t.append(h)

            # ---- relu(x) on ACT (parallel with DVE prep)
            xp = []
            for t in range(NT):
                xpt = work.tile([128, BS], bf16, tag=f"xp{t}")
                nc.scalar.activation(xpt, x_sb[t], AF.Relu)
                xp.append(xpt)

            # ---- matmuls: psum half = batch half; weight-tile-0 matmuls
            # for both halves first so tile-1 prep overlaps them.
            psum = []
            for h in range(2):
                ps = pp.tile([128, HB], f32, tag=f"ps{h}")
                psum.append(ps)
            for t in range(NT):
                for h in range(2):
                    sl = slice(h * HB, (h + 1) * HB)
                    nc.tensor.matmul(psum[h], lhsT=g_t[t], rhs=xp[t][:, sl],
                                     start=(t == 0), stop=False)
                    nc.tensor.matmul(psum[h], lhsT=h_t[t], rhs=x_sb[t][:, sl],
                                     start=False, stop=(t == NT - 1))

            # ---- epilogue + store per half
            for h in range(2):
                out_sb = work.tile([128, HB], bf16, tag=f"out{h}")
                nc.scalar.activation(out_sb, psum[h], AF.Identity,
                                     bias=b2_sb, scale=float(SCALE))
                nc.sync.dma_start(out=outt[:, h * HB:(h + 1) * HB], in_=out_sb)

    nc.compile()
    _CACHE["nc"] = nc
    return nc


def _kan_numpy(x, w1, b1, w2, b2):
    # exact fallback, chunked over batch to bound memory
    out = np.empty((x.shape[0], w1.shape[0]), dtype=np.float32)
    d = (w1.shape[0] + w1.shape[1]) / 2
    s = 1.0 / np.sqrt(d * w1.shape[2])
    for lo in range(0, x.shape[0], 128):
        hi = min(lo + 128, x.shape[0])
        h = x[lo:hi, None, :, None] * w1[None] + b1[None]
        np.maximum(h, 0.0, out=h)
        out[lo:hi] = np.einsum("boik,oik->bo", h, w2) * s
    return out + b2[None, :]


def kernel(x, w1, b1, w2, b2):
    x = np.asarray(x, dtype=np.float32)
    w1 = np.asarray(w1, dtype=np.float32)
    b1 = np.asarray(b1, dtype=np.float32)
    w2 = np.asarray(w2, dtype=np.float32)
    b2 = np.asarray(b2, dtype=np.float32)

    if x.shape != (B, DIN) or w1.shape != (DOUT, DIN, K) or np.any(b1):
        return _kan_numpy(x, w1, b1, w2, b2)

    import ml_dtypes
    from concourse.bass_utils import run_bass_kernel_spmd

    nc = _build_nc()
    bf16 = ml_dtypes.bfloat16

    xT = np.ascontiguousarray(x.T).astype(bf16)          # (DIN, B)
    w1T = w1.transpose(1, 2, 0).astype(bf16)             # (DIN, K, DOUT)
    w2T = w2.transpose(1, 2, 0).astype(bf16)

    in_maps = []
    for core in range(N_CORES):
        bi, oj = divmod(core, OG)
        osl = slice(oj * OS, (oj + 1) * OS)
        wtc = np.empty((128, NT, 2, K, OS), dtype=bf16)
        for t in range(NT):
            isl = slice(t * 128, (t + 1) * 128)
            wtc[:, t, 0] = w1T[isl, :, osl]
            wtc[:, t, 1] = w2T[isl, :, osl]
        xtc = np.empty((128, NT, BS), dtype=bf16)
        for t in range(NT):
            xtc[:, t] = xT[t * 128:(t + 1) * 128, bi * BS:(bi + 1) * BS]
        in_maps.append({
            "wt": wtc,
            "xt": xtc,
            "b2s": np.ascontiguousarray(b2[osl], dtype=np.float32).reshape(OS, 1),
        })

    res = run_bass_kernel_spmd(nc, in_maps, core_ids=list(range(N_CORES)))

    out = np.empty((B, DOUT), dtype=np.float32)
    for core in range(N_CORES):
        bi, oj = divmod(core, OG)
        out[bi * BS:(bi + 1) * BS, oj * OS:(oj + 1) * OS] = \
            res.results[core]["outt"].astype(np.float32).T
    return out
